# revision 14
# baseline (speedup 1.0000x reference)
"""Bidirectional-LSTM (degenerate variant) Trainium2 kernel.

Reference semantics (see harness): for the forward direction only the last
timestep matters (h/c never update), and the backward direction is an
h-only recurrence (c stays zero), so only the i/g/o gates are ever used:

    h_fwd = sig(o) * tanh(sig(i) * tanh(g)),  gates = x_last @ W_ih_f.T + b_f
    h_bwd: scan t = S-1..0 with
        gates = x_t @ W_ih_b.T + b_b + h @ W_hh_b.T   (f-gate unused)
        h     = sig(o) * tanh(sig(i) * tanh(g))
    out = [h_fwd | h_bwd]  -> [256, 4096]

Distribution: pure data-parallel over batch (32 per core, 8 cores), weights
replicated. Per core:
  pass A : embedding gather (indirect DMA) + PE-transpose of X -> XT in DRAM
  phase 1: input projection xg = X @ Wi + b in fp32r (full fp32 inputs),
           stored bf16; forward cell folded in
  phase R: 128-step recurrence. gates = Wr.T @ h via 4 col-tiled concurrent
           M=32 matmuls (bf16), + xg, activations, PE-transpose of h for the
           next step's stationary operand.

Gate columns are host-permuted into 4 groups of (i|g|o) x 512 hid dims so
each PSUM column-group j directly yields h[:, 512j:512j+512].
"""

import numpy as np
import ml_dtypes

import concourse.bass as bass
import concourse.bacc as bacc
import concourse.mybir as mybir
import concourse.tile as tile
from concourse.masks import make_identity

VOCAB, EMB, HID = 50000, 1024, 2048
BATCH, SEQ = 256, 128
NCORES = 8
BLOC = BATCH // NCORES            # 32 batch rows per core
NTOK = BLOC * SEQ                 # 4096 tokens per core
NG = 4                            # PSUM column groups
GC = 3 * HID // NG                # 1536 gate cols per group (i|g|o x 512)
HG = HID // NG                    # 512 hid dims per group
G3 = 3 * HID                      # 6144 total igo gate cols
MT = NTOK // 128                  # 32 token m-tiles
KT_E = EMB // 128                 # 8 k-tiles for input projection
KT_H = HID // 128                 # 16 k-tiles for recurrence

F32 = mybir.dt.float32
F32R = mybir.dt.float32r
BF16 = mybir.dt.bfloat16
I32 = mybir.dt.int32

N_STEPS = SEQ  # overridable for mini builds


def build(n_steps=None):
    n_steps = n_steps or N_STEPS
    nc = bacc.Bacc("TRN2", target_bir_lowering=False, debug=False,
                   num_devices=NCORES)

    tok = nc.dram_tensor("tok", [NTOK, 1], I32, kind="ExternalInput")
    table = nc.dram_tensor("table", [VOCAB, EMB], F32R, kind="ExternalInput")
    Wi = nc.dram_tensor("Wi", [EMB, G3], F32R, kind="ExternalInput")
    Wf = nc.dram_tensor("Wf", [EMB, G3], F32R, kind="ExternalInput")
    Wr = nc.dram_tensor("Wr", [HID, G3], BF16, kind="ExternalInput")
    bias_b = nc.dram_tensor("bias_b", [128, G3], F32, kind="ExternalInput")
    bias_f = nc.dram_tensor("bias_f", [128, G3], F32, kind="ExternalInput")
    identf = nc.dram_tensor("identf", [128, 128], F32R, kind="ExternalInput")
    out = nc.dram_tensor("out", [BLOC, 2 * HID], F32, kind="ExternalOutput")

    XTd = nc.dram_tensor("XTd", [MT, 128, EMB], F32R)      # internal
    xgd = nc.dram_tensor("xgd", [NTOK, G3], BF16)         # internal

    with tile.TileContext(nc) as tc:
        # ---------------- pass A: gather + transpose ----------------
        with tc.tile_pool(name="pa", bufs=2) as pa, \
             tc.tile_pool(name="pa1", bufs=1) as pa1, \
             tc.tile_pool(name="pa_ps", bufs=4, space="PSUM") as pa_ps:
            ident = pa1.tile([128, 128], F32R)
            nc.sync.dma_start(out=ident[:], in_=identf[:, :])
            for m in range(MT):
                idx_sb = pa.tile([128, 1], I32, tag="idx")
                nc.sync.dma_start(out=idx_sb[:], in_=tok[m * 128:(m + 1) * 128, :])
                x_sb = pa.tile([128, EMB], F32R, tag="x")
                nc.gpsimd.indirect_dma_start(
                    out=x_sb[:], out_offset=None, in_=table[:, :],
                    in_offset=bass.IndirectOffsetOnAxis(ap=idx_sb[:, :1], axis=0))
                xt_sb = pa.tile([128, EMB], F32R, tag="xt")
                for q in range(KT_E):
                    t_ps = pa_ps.tile([128, 128], F32R, space="PSUM", tag="tps")
                    nc.tensor.transpose(out=t_ps[:], in_=x_sb[:, 128 * q:128 * (q + 1)],
                                        identity=ident[:])
                    nc.vector.tensor_copy(xt_sb[:, 128 * q:128 * (q + 1)], t_ps[:])
                nc.sync.dma_start(out=XTd[m, :, :], in_=xt_sb[:])

        tc.strict_bb_all_engine_barrier()
        # ---------------- phase 1: input projection ----------------
        with tc.tile_pool(name="p1w", bufs=2) as p1w, \
             tc.tile_pool(name="p1wf", bufs=1) as p1wf, \
             tc.tile_pool(name="p1", bufs=2) as p1, \
             tc.tile_pool(name="p1s", bufs=1) as p1s, \
             tc.tile_pool(name="p1_ps", bufs=2, space="PSUM") as p1_ps:
            xt0_sb = p1s.tile([128, EMB], F32R)
            nc.sync.dma_start(out=xt0_sb[:], in_=XTd[0, :, :])
            for blk in range(NG):
                cs = slice(GC * blk, GC * (blk + 1))
                wi_sb = p1w.tile([128, KT_E, GC], F32R, tag="wi")
                nc.sync.dma_start(
                    out=wi_sb[:],
                    in_=Wi[:, cs].rearrange("(k p) c -> p k c", p=128))
                wf_sb = p1wf.tile([128, KT_E, GC], F32R, tag="wf")
                nc.sync.dma_start(
                    out=wf_sb[:],
                    in_=Wf[:, cs].rearrange("(k p) c -> p k c", p=128))
                bia_sb = p1.tile([128, GC], F32, tag="bia")
                nc.sync.dma_start(out=bia_sb[:], in_=bias_b[:, cs])
                for m in range(MT):
                    xt_sb = p1.tile([128, EMB], F32R, tag="xtl")
                    nc.sync.dma_start(out=xt_sb[:], in_=XTd[m, :, :])
                    ps = p1_ps.tile([128, GC], F32, space="PSUM", tag="ps")
                    for c in range(3):
                        for k in range(KT_E):
                            nc.tensor.matmul(
                                ps[:, 512 * c:512 * (c + 1)],
                                lhsT=xt_sb[:, 128 * k:128 * (k + 1)],
                                rhs=wi_sb[:, k, 512 * c:512 * (c + 1)],
                                start=(k == 0), stop=(k == KT_E - 1))
                    xg_sb = p1.tile([128, GC], BF16, tag="xg")
                    nc.vector.tensor_add(xg_sb[:], ps[:], bia_sb[:])
                    nc.sync.dma_start(out=xgd[m * 128:(m + 1) * 128, cs], in_=xg_sb[:])
                # forward cell for this block (tokens 0..32 = original last step)
                psf = p1_ps.tile([128, GC], F32, space="PSUM", tag="ps")
                for c in range(3):
                    for k in range(KT_E):
                        nc.tensor.matmul(
                            psf[0:BLOC, 512 * c:512 * (c + 1)],
                            lhsT=xt0_sb[:, 128 * k:128 * k + BLOC],
                            rhs=wf_sb[:, k, 512 * c:512 * (c + 1)],
                            start=(k == 0), stop=(k == KT_E - 1))
                bif_sb = p1s.tile([BLOC, GC], F32, tag="bif")
                nc.sync.dma_start(out=bif_sb[:], in_=bias_f[0:BLOC, cs])
                gf = p1s.tile([BLOC, GC], F32, tag="gf")
                nc.vector.tensor_add(gf[:], psf[0:BLOC, :], bif_sb[:])
                af = p1s.tile([BLOC, HG], F32, tag="af")
                bf = p1s.tile([BLOC, HG], F32, tag="bff")
                cf = p1s.tile([BLOC, HG], F32, tag="cf")
                nc.scalar.activation(af[:], gf[:, 0:HG],
                                     mybir.ActivationFunctionType.Sigmoid)
                nc.scalar.activation(bf[:], gf[:, HG:2 * HG],
                                     mybir.ActivationFunctionType.Tanh)
                nc.scalar.activation(cf[:], gf[:, 2 * HG:3 * HG],
                                     mybir.ActivationFunctionType.Sigmoid)
                nc.vector.tensor_mul(af[:], af[:], bf[:])
                nc.scalar.activation(af[:], af[:],
                                     mybir.ActivationFunctionType.Tanh)
                nc.vector.tensor_mul(af[:], cf[:], af[:])
                nc.sync.dma_start(out=out[:, HG * blk:HG * (blk + 1)], in_=af[:])

        tc.strict_bb_all_engine_barrier()
        # ---------------- phase R: recurrence ----------------
        with tc.tile_pool(name="prw", bufs=1) as prw, \
             tc.tile_pool(name="pr", bufs=2) as pr, \
             tc.tile_pool(name="pr1", bufs=1) as pr1, \
             tc.tile_pool(name="prh", bufs=2) as prh, \
             tc.tile_pool(name="pr_ps", bufs=2, space="PSUM") as pr_ps, \
             tc.tile_pool(name="prt_ps", bufs=2, space="PSUM") as prt_ps:
            wr_sb = prw.tile([128, KT_H, G3], BF16)
            nc.sync.dma_start(
                out=wr_sb[:], in_=Wr[:, :].rearrange("(k p) c -> p k c", p=128))
            identb = pr1.tile([128, 128], BF16)
            make_identity(nc, identb[:])

            a_t = pr1.tile([128, HG], F32)
            b_t = pr1.tile([128, HG], F32)

            def load_xg(s):
                xg_sb = pr.tile([128, GC], BF16, tag="xgs")
                for j in range(NG):
                    nc.sync.dma_start(
                        out=xg_sb[BLOC * j:BLOC * (j + 1), :],
                        in_=xgd[BLOC * s:BLOC * (s + 1), GC * j:GC * (j + 1)])
                return xg_sb

            def act_and_transpose(gates_ap, store_out=False):
                """gates_ap: [128,(i|g|o)*HG] fp32/bf16 readable; returns hT tile."""
                nc.scalar.activation(a_t[:], gates_ap[:, 0:HG],
                                     mybir.ActivationFunctionType.Sigmoid)
                nc.scalar.activation(b_t[:], gates_ap[:, HG:2 * HG],
                                     mybir.ActivationFunctionType.Tanh)
                nc.vector.tensor_mul(a_t[:], a_t[:], b_t[:])      # u = sig(i)*tanh(g)
                nc.scalar.activation(a_t[:], a_t[:],
                                     mybir.ActivationFunctionType.Tanh)  # v
                nc.scalar.activation(b_t[:], gates_ap[:, 2 * HG:3 * HG],
                                     mybir.ActivationFunctionType.Sigmoid)  # c
                h_t = pr.tile([128, HG], F32 if store_out else BF16, tag="h",
                              bufs=1)
                nc.vector.tensor_mul(h_t[:], b_t[:], a_t[:])
                if store_out:
                    for j in range(NG):
                        nc.sync.dma_start(
                            out=out[:, HID + HG * j:HID + HG * (j + 1)],
                            in_=h_t[BLOC * j:BLOC * (j + 1), :])
                    return None
                hT = prh.tile([128, HG], BF16, tag="hT")
                t_ps = prt_ps.tile([128, HG], BF16, space="PSUM", tag="tps")
                for q in range(NG):
                    nc.tensor.transpose(out=t_ps[:, 128 * q:128 * (q + 1)],
                                        in_=h_t[:, 128 * q:128 * (q + 1)],
                                        identity=identb[:])
                nc.vector.tensor_copy(hT[:], t_ps[:])
                return hT

            # step 0: h=0 -> gates are just xg
            xg0 = load_xg(0)
            hT = act_and_transpose(xg0[:])

            for s in range(1, n_steps):
                xg_sb = load_xg(s)
                ps_g = pr_ps.tile([128, GC], F32, space="PSUM", tag="gps")
                for c in range(3):
                    for k in range(KT_H):
                        lhs = hT[:, 128 * (k % NG) + BLOC * (k // NG):
                                 128 * (k % NG) + BLOC * (k // NG) + BLOC]
                        for j in range(NG):
                            nc.tensor.matmul(
                                ps_g[BLOC * j:BLOC * (j + 1), 512 * c:512 * (c + 1)],
                                lhsT=lhs,
                                rhs=wr_sb[:, k, GC * j + 512 * c:GC * j + 512 * (c + 1)],
                                start=(k == 0), stop=(k == KT_H - 1),
                                tile_position=(0, BLOC * j),
                                skip_group_check=True)
                nc.vector.tensor_add(ps_g[:], ps_g[:], xg_sb[:])
                hT = act_and_transpose(ps_g[:], store_out=(s == n_steps - 1))
    nc.compile()
    return nc


_BUILT = {}


def _get_built(n_steps=None):
    key = n_steps or N_STEPS
    if key not in _BUILT:
        _BUILT[key] = build(key)
    return _BUILT[key]


def _perm():
    """Row permutation taking PyTorch (i|f|g|o)*2048 rows to 4 groups of
    (i|g|o)*512."""
    p = []
    for j in range(NG):
        for base in (0, 2 * HID, 3 * HID):  # i, g, o blocks
            p.extend(range(base + HG * j, base + HG * (j + 1)))
    return np.array(p)


def prep_inputs(inputs, embed_table, W_ih_f, W_hh_f, b_ih_f, b_hh_f,
                W_ih_b, W_hh_b, b_ih_b, b_hh_b):
    perm = _perm()
    idx = np.asarray(inputs)
    idx = np.where(idx > VOCAB, 0, idx).astype(np.int64)
    idx = np.clip(idx, 0, VOCAB - 1).astype(np.int32)

    Wi_p = np.ascontiguousarray(np.asarray(W_ih_b)[perm].T.astype(np.float32))
    Wf_p = np.ascontiguousarray(np.asarray(W_ih_f)[perm].T.astype(np.float32))
    Wr_p = np.ascontiguousarray(
        np.asarray(W_hh_b)[perm].T.astype(ml_dtypes.bfloat16))
    bb = (np.asarray(b_ih_b) + np.asarray(b_hh_b))[perm].astype(np.float32)
    bf = (np.asarray(b_ih_f) + np.asarray(b_hh_f))[perm].astype(np.float32)
    bias_b_t = np.ascontiguousarray(np.broadcast_to(bb, (128, G3)))
    bias_f_t = np.ascontiguousarray(np.broadcast_to(bf, (128, G3)))
    table = np.ascontiguousarray(np.asarray(embed_table, dtype=np.float32))
    identf = np.eye(128, dtype=np.float32)

    in_maps = []
    for c in range(NCORES):
        sl = idx[BLOC * c:BLOC * (c + 1)]          # [32, 128]
        tok = np.ascontiguousarray(sl[:, ::-1].T.reshape(NTOK, 1))  # t-major rev
        in_maps.append({
            "tok": tok, "table": table, "Wi": Wi_p, "Wf": Wf_p, "Wr": Wr_p,
            "bias_b": bias_b_t, "bias_f": bias_f_t, "identf": identf,
        })
    return in_maps


def kernel(**inputs) -> np.ndarray:
    from concourse.bass_utils import run_bass_kernel_spmd
    nc = _get_built()
    in_maps = prep_inputs(**inputs)
    res = run_bass_kernel_spmd(nc, in_maps, core_ids=list(range(NCORES)))
    return np.concatenate([res.results[c]["out"] for c in range(NCORES)], axis=0)


# revision 17
# speedup vs baseline: 1.0035x; 1.0035x over previous
"""Bidirectional-LSTM (degenerate variant) Trainium2 kernel.

Reference semantics (see harness): for the forward direction only the last
timestep matters (h/c never update), and the backward direction is an
h-only recurrence (c stays zero), so only the i/g/o gates are ever used:

    h_fwd = sig(o) * tanh(sig(i) * tanh(g)),  gates = x_last @ W_ih_f.T + b_f
    h_bwd: scan t = S-1..0 with
        gates = x_t @ W_ih_b.T + b_b + h @ W_hh_b.T   (f-gate unused)
        h     = sig(o) * tanh(sig(i) * tanh(g))
    out = [h_fwd | h_bwd]  -> [256, 4096]

Distribution: pure data-parallel over batch (32 per core, 8 cores), weights
replicated. Per core:
  pass A : embedding gather (indirect DMA) + PE-transpose of X -> XT in DRAM
  phase 1: input projection xg = X @ Wi + b in fp32r (full fp32 inputs),
           stored bf16; forward cell folded in
  phase R: 128-step recurrence. gates = Wr.T @ h via 4 col-tiled concurrent
           M=32 matmuls (bf16), + xg, activations, PE-transpose of h for the
           next step's stationary operand.

Gate columns are host-permuted into 4 groups of (i|g|o) x 512 hid dims so
each PSUM column-group j directly yields h[:, 512j:512j+512].
"""

import numpy as np
import ml_dtypes

import concourse.bass as bass
import concourse.bacc as bacc
import concourse.mybir as mybir
import concourse.tile as tile
from concourse.masks import make_identity

VOCAB, EMB, HID = 50000, 1024, 2048
BATCH, SEQ = 256, 128
NCORES = 8
BLOC = BATCH // NCORES            # 32 batch rows per core
NTOK = BLOC * SEQ                 # 4096 tokens per core
NG = 4                            # PSUM column groups
GC = 3 * HID // NG                # 1536 gate cols per group (i|g|o x 512)
HG = HID // NG                    # 512 hid dims per group
G3 = 3 * HID                      # 6144 total igo gate cols
MT = NTOK // 128                  # 32 token m-tiles
KT_E = EMB // 128                 # 8 k-tiles for input projection
KT_H = HID // 128                 # 16 k-tiles for recurrence

F32 = mybir.dt.float32
F32R = mybir.dt.float32r
BF16 = mybir.dt.bfloat16
I32 = mybir.dt.int32

N_STEPS = SEQ  # overridable for mini builds


def build(n_steps=None):
    n_steps = n_steps or N_STEPS
    nc = bacc.Bacc("TRN2", target_bir_lowering=False, debug=False,
                   num_devices=NCORES)

    tok = nc.dram_tensor("tok", [NTOK, 1], I32, kind="ExternalInput")
    table = nc.dram_tensor("table", [VOCAB, EMB], F32R, kind="ExternalInput")
    Wi = nc.dram_tensor("Wi", [EMB, G3], F32R, kind="ExternalInput")
    Wf = nc.dram_tensor("Wf", [EMB, G3], F32R, kind="ExternalInput")
    Wr = nc.dram_tensor("Wr", [HID, G3], BF16, kind="ExternalInput")
    bias_b = nc.dram_tensor("bias_b", [128, G3], F32, kind="ExternalInput")
    bias_f = nc.dram_tensor("bias_f", [128, G3], F32, kind="ExternalInput")
    identf = nc.dram_tensor("identf", [128, 128], F32R, kind="ExternalInput")
    out = nc.dram_tensor("out", [BLOC, 2 * HID], F32, kind="ExternalOutput")

    XTd = nc.dram_tensor("XTd", [MT, 128, EMB], F32R)      # internal
    xgd = nc.dram_tensor("xgd", [NTOK, G3], BF16)         # internal

    with tile.TileContext(nc) as tc:
        # ---------------- pass A: gather + transpose ----------------
        with tc.tile_pool(name="pa", bufs=2) as pa, \
             tc.tile_pool(name="pa1", bufs=1) as pa1, \
             tc.tile_pool(name="pa_ps", bufs=4, space="PSUM") as pa_ps:
            ident = pa1.tile([128, 128], F32R)
            nc.sync.dma_start(out=ident[:], in_=identf[:, :])
            for m in range(MT):
                idx_sb = pa.tile([128, 1], I32, tag="idx")
                nc.sync.dma_start(out=idx_sb[:], in_=tok[m * 128:(m + 1) * 128, :])
                x_sb = pa.tile([128, EMB], F32R, tag="x")
                nc.gpsimd.indirect_dma_start(
                    out=x_sb[:], out_offset=None, in_=table[:, :],
                    in_offset=bass.IndirectOffsetOnAxis(ap=idx_sb[:, :1], axis=0))
                xt_sb = pa.tile([128, EMB], F32R, tag="xt")
                for q in range(KT_E):
                    t_ps = pa_ps.tile([128, 128], F32R, space="PSUM", tag="tps")
                    nc.tensor.transpose(out=t_ps[:], in_=x_sb[:, 128 * q:128 * (q + 1)],
                                        identity=ident[:])
                    nc.vector.tensor_copy(xt_sb[:, 128 * q:128 * (q + 1)], t_ps[:])
                nc.sync.dma_start(out=XTd[m, :, :], in_=xt_sb[:])

        tc.strict_bb_all_engine_barrier()
        # ---------------- phase 1: input projection ----------------
        with tc.tile_pool(name="p1w", bufs=2) as p1w, \
             tc.tile_pool(name="p1wf", bufs=1) as p1wf, \
             tc.tile_pool(name="p1", bufs=2) as p1, \
             tc.tile_pool(name="p1s", bufs=1) as p1s, \
             tc.tile_pool(name="p1_ps", bufs=2, space="PSUM") as p1_ps:
            xt0_sb = p1s.tile([128, EMB], F32R)
            nc.sync.dma_start(out=xt0_sb[:], in_=XTd[0, :, :])
            for blk in range(NG):
                cs = slice(GC * blk, GC * (blk + 1))
                wi_sb = p1w.tile([128, KT_E, GC], F32R, tag="wi")
                nc.sync.dma_start(
                    out=wi_sb[:],
                    in_=Wi[:, cs].rearrange("(k p) c -> p k c", p=128))
                wf_sb = p1wf.tile([128, KT_E, GC], F32R, tag="wf")
                nc.sync.dma_start(
                    out=wf_sb[:],
                    in_=Wf[:, cs].rearrange("(k p) c -> p k c", p=128))
                bia_sb = p1.tile([128, GC], F32, tag="bia")
                nc.sync.dma_start(out=bia_sb[:], in_=bias_b[:, cs])
                for m in range(MT):
                    xt_sb = p1.tile([128, EMB], F32R, tag="xtl")
                    nc.sync.dma_start(out=xt_sb[:], in_=XTd[m, :, :])
                    ps = p1_ps.tile([128, GC], F32, space="PSUM", tag="ps")
                    for c in range(3):
                        for k in range(KT_E):
                            nc.tensor.matmul(
                                ps[:, 512 * c:512 * (c + 1)],
                                lhsT=xt_sb[:, 128 * k:128 * (k + 1)],
                                rhs=wi_sb[:, k, 512 * c:512 * (c + 1)],
                                start=(k == 0), stop=(k == KT_E - 1))
                    xg_sb = p1.tile([128, GC], BF16, tag="xg")
                    nc.vector.tensor_add(xg_sb[:], ps[:], bia_sb[:])
                    nc.sync.dma_start(out=xgd[m * 128:(m + 1) * 128, cs], in_=xg_sb[:])
                # forward cell for this block (tokens 0..32 = original last step)
                psf = p1_ps.tile([128, GC], F32, space="PSUM", tag="ps")
                for c in range(3):
                    for k in range(KT_E):
                        nc.tensor.matmul(
                            psf[0:BLOC, 512 * c:512 * (c + 1)],
                            lhsT=xt0_sb[:, 128 * k:128 * k + BLOC],
                            rhs=wf_sb[:, k, 512 * c:512 * (c + 1)],
                            start=(k == 0), stop=(k == KT_E - 1))
                bif_sb = p1s.tile([BLOC, GC], F32, tag="bif")
                nc.sync.dma_start(out=bif_sb[:], in_=bias_f[0:BLOC, cs])
                gf = p1s.tile([BLOC, GC], F32, tag="gf")
                nc.vector.tensor_add(gf[:], psf[0:BLOC, :], bif_sb[:])
                af = p1s.tile([BLOC, HG], F32, tag="af")
                bf = p1s.tile([BLOC, HG], F32, tag="bff")
                cf = p1s.tile([BLOC, HG], F32, tag="cf")
                nc.scalar.activation(af[:], gf[:, 0:HG],
                                     mybir.ActivationFunctionType.Sigmoid)
                nc.scalar.activation(bf[:], gf[:, HG:2 * HG],
                                     mybir.ActivationFunctionType.Tanh)
                nc.scalar.activation(cf[:], gf[:, 2 * HG:3 * HG],
                                     mybir.ActivationFunctionType.Sigmoid)
                nc.vector.tensor_mul(af[:], af[:], bf[:])
                nc.scalar.activation(af[:], af[:],
                                     mybir.ActivationFunctionType.Tanh)
                nc.vector.tensor_mul(af[:], cf[:], af[:])
                nc.sync.dma_start(out=out[:, HG * blk:HG * (blk + 1)], in_=af[:])

        tc.strict_bb_all_engine_barrier()
        # ---------------- phase R: recurrence ----------------
        with tc.tile_pool(name="prw", bufs=1) as prw, \
             tc.tile_pool(name="pr", bufs=2) as pr, \
             tc.tile_pool(name="pr1", bufs=1) as pr1, \
             tc.tile_pool(name="prh", bufs=8) as prh, \
             tc.tile_pool(name="pr_ps", bufs=2, space="PSUM") as pr_ps, \
             tc.tile_pool(name="prt_ps", bufs=2, space="PSUM") as prt_ps:
            wr_sb = prw.tile([128, KT_H, G3], BF16)
            nc.sync.dma_start(
                out=wr_sb[:], in_=Wr[:, :].rearrange("(k p) c -> p k c", p=128))
            identb = pr1.tile([128, 128], BF16)
            make_identity(nc, identb[:])

            a_t = pr1.tile([128, HG], F32)
            b_t = pr1.tile([128, HG], F32)

            def load_xg(s):
                xg_sb = pr.tile([128, GC], BF16, tag="xgs")
                for j in range(NG):
                    nc.sync.dma_start(
                        out=xg_sb[BLOC * j:BLOC * (j + 1), :],
                        in_=xgd[BLOC * s:BLOC * (s + 1), GC * j:GC * (j + 1)])
                return xg_sb

            def act_and_transpose(gates_ap, store_out=False):
                """gates_ap: [128,(i|g|o)*HG] fp32/bf16 readable; returns list of
                4 hT chunk tiles [128,128] (hT[c][:, 32j:32j+32] = k-tile 4j+c)."""
                nc.scalar.activation(a_t[:], gates_ap[:, 0:HG],
                                     mybir.ActivationFunctionType.Sigmoid)
                nc.scalar.activation(b_t[:], gates_ap[:, HG:2 * HG],
                                     mybir.ActivationFunctionType.Tanh)
                nc.vector.tensor_mul(a_t[:], a_t[:], b_t[:])      # u = sig(i)*tanh(g)
                nc.scalar.activation(a_t[:], a_t[:],
                                     mybir.ActivationFunctionType.Tanh)  # v
                nc.scalar.activation(b_t[:], gates_ap[:, 2 * HG:3 * HG],
                                     mybir.ActivationFunctionType.Sigmoid)  # c
                if store_out:
                    h_t = pr.tile([128, HG], F32, tag="hfin", bufs=1)
                    nc.vector.tensor_mul(h_t[:], b_t[:], a_t[:])
                    for j in range(NG):
                        nc.sync.dma_start(
                            out=out[:, HID + HG * j:HID + HG * (j + 1)],
                            in_=h_t[BLOC * j:BLOC * (j + 1), :])
                    return None
                # chunk-pipelined: mul -> PE transpose -> copy per 128-col chunk
                hTs = []
                for q in range(NG):
                    h_q = pr.tile([128, 128], BF16, tag="h", bufs=3)
                    nc.vector.tensor_mul(h_q[:], b_t[:, 128 * q:128 * (q + 1)],
                                         a_t[:, 128 * q:128 * (q + 1)])
                    t_ps = prt_ps.tile([128, 128], BF16, space="PSUM", tag="tps")
                    nc.tensor.transpose(out=t_ps[:], in_=h_q[:],
                                        identity=identb[:])
                    hT_q = prh.tile([128, 128], BF16, tag="hT")
                    nc.vector.tensor_copy(hT_q[:], t_ps[:])
                    hTs.append(hT_q)
                return hTs

            # step 0: h=0 -> gates are just xg
            xg0 = load_xg(0)
            hT = act_and_transpose(xg0[:])

            for s in range(1, n_steps):
                xg_sb = load_xg(s)
                ps_g = pr_ps.tile([128, GC], F32, space="PSUM", tag="gps")
                for c in range(3):
                    for k in range(KT_H):
                        lhs = hT[k % NG][:, BLOC * (k // NG):BLOC * (k // NG) + BLOC]
                        for j in range(NG):
                            nc.tensor.matmul(
                                ps_g[BLOC * j:BLOC * (j + 1), 512 * c:512 * (c + 1)],
                                lhsT=lhs,
                                rhs=wr_sb[:, k, GC * j + 512 * c:GC * j + 512 * (c + 1)],
                                start=(k == 0), stop=(k == KT_H - 1),
                                tile_position=(0, BLOC * j),
                                skip_group_check=True)
                    # fold xg into this bank as soon as its accumulation is done
                    nc.vector.tensor_add(
                        ps_g[:, 512 * c:512 * (c + 1)],
                        ps_g[:, 512 * c:512 * (c + 1)],
                        xg_sb[:, 512 * c:512 * (c + 1)])
                hT = act_and_transpose(ps_g[:], store_out=(s == n_steps - 1))
    nc.compile()
    return nc


_BUILT = {}


def _get_built(n_steps=None):
    key = n_steps or N_STEPS
    if key not in _BUILT:
        _BUILT[key] = build(key)
    return _BUILT[key]


def _perm():
    """Row permutation taking PyTorch (i|f|g|o)*2048 rows to 4 groups of
    (i|g|o)*512."""
    p = []
    for j in range(NG):
        for base in (0, 2 * HID, 3 * HID):  # i, g, o blocks
            p.extend(range(base + HG * j, base + HG * (j + 1)))
    return np.array(p)


def prep_inputs(inputs, embed_table, W_ih_f, W_hh_f, b_ih_f, b_hh_f,
                W_ih_b, W_hh_b, b_ih_b, b_hh_b):
    perm = _perm()
    idx = np.asarray(inputs)
    idx = np.where(idx > VOCAB, 0, idx).astype(np.int64)
    idx = np.clip(idx, 0, VOCAB - 1).astype(np.int32)

    Wi_p = np.ascontiguousarray(np.asarray(W_ih_b)[perm].T.astype(np.float32))
    Wf_p = np.ascontiguousarray(np.asarray(W_ih_f)[perm].T.astype(np.float32))
    Wr_p = np.ascontiguousarray(
        np.asarray(W_hh_b)[perm].T.astype(ml_dtypes.bfloat16))
    bb = (np.asarray(b_ih_b) + np.asarray(b_hh_b))[perm].astype(np.float32)
    bf = (np.asarray(b_ih_f) + np.asarray(b_hh_f))[perm].astype(np.float32)
    bias_b_t = np.ascontiguousarray(np.broadcast_to(bb, (128, G3)))
    bias_f_t = np.ascontiguousarray(np.broadcast_to(bf, (128, G3)))
    table = np.ascontiguousarray(np.asarray(embed_table, dtype=np.float32))
    identf = np.eye(128, dtype=np.float32)

    in_maps = []
    for c in range(NCORES):
        sl = idx[BLOC * c:BLOC * (c + 1)]          # [32, 128]
        tok = np.ascontiguousarray(sl[:, ::-1].T.reshape(NTOK, 1))  # t-major rev
        in_maps.append({
            "tok": tok, "table": table, "Wi": Wi_p, "Wf": Wf_p, "Wr": Wr_p,
            "bias_b": bias_b_t, "bias_f": bias_f_t, "identf": identf,
        })
    return in_maps


def kernel(**inputs) -> np.ndarray:
    from concourse.bass_utils import run_bass_kernel_spmd
    nc = _get_built()
    in_maps = prep_inputs(**inputs)
    res = run_bass_kernel_spmd(nc, in_maps, core_ids=list(range(NCORES)))
    return np.concatenate([res.results[c]["out"] for c in range(NCORES)], axis=0)


# revision 20
# speedup vs baseline: 1.3023x; 1.2978x over previous
"""Bidirectional-LSTM (degenerate variant) Trainium2 kernel.

Reference semantics (see harness): for the forward direction only the last
timestep matters (h/c never update), and the backward direction is an
h-only recurrence (c stays zero), so only the i/g/o gates are ever used:

    h_fwd = sig(o) * tanh(sig(i) * tanh(g)),  gates = x_last @ W_ih_f.T + b_f
    h_bwd: scan t = S-1..0 with
        gates = x_t @ W_ih_b.T + b_b + h @ W_hh_b.T   (f-gate unused)
        h     = sig(o) * tanh(sig(i) * tanh(g))
    out = [h_fwd | h_bwd]  -> [256, 4096]

Distribution: pure data-parallel over batch (32 per core, 8 cores), weights
replicated. Per core:
  pass A : embedding gather (indirect DMA) + PE-transpose of X -> XT in DRAM
  phase 1: input projection xg = X @ Wi + b in fp32r (full fp32 inputs),
           stored bf16; forward cell folded in
  phase R: 128-step recurrence. gates = Wr.T @ h via 4 col-tiled concurrent
           M=32 matmuls (bf16), + xg, activations, PE-transpose of h for the
           next step's stationary operand.

Gate columns are host-permuted into 4 groups of (i|g|o) x 512 hid dims so
each PSUM column-group j directly yields h[:, 512j:512j+512].
"""

import numpy as np
import ml_dtypes

import concourse.bass as bass
import concourse.bacc as bacc
import concourse.mybir as mybir
import concourse.tile as tile
from concourse.masks import make_identity

VOCAB, EMB, HID = 50000, 1024, 2048
BATCH, SEQ = 256, 128
NCORES = 8
BLOC = BATCH // NCORES            # 32 batch rows per core
NTOK = BLOC * SEQ                 # 4096 tokens per core
NG = 4                            # PSUM column groups
GC = 3 * HID // NG                # 1536 gate cols per group (i|g|o x 512)
HG = HID // NG                    # 512 hid dims per group
G3 = 3 * HID                      # 6144 total igo gate cols
MT = NTOK // 128                  # 32 token m-tiles
KT_E = EMB // 128                 # 8 k-tiles for input projection
KT_H = HID // 128                 # 16 k-tiles for recurrence

F32 = mybir.dt.float32
F32R = mybir.dt.float32r
BF16 = mybir.dt.bfloat16
I32 = mybir.dt.int32

N_STEPS = SEQ  # overridable for mini builds


def build(n_steps=None):
    n_steps = n_steps or N_STEPS
    nc = bacc.Bacc("TRN2", target_bir_lowering=False, debug=False,
                   num_devices=NCORES)

    tok = nc.dram_tensor("tok", [NTOK, 1], I32, kind="ExternalInput")
    table = nc.dram_tensor("table", [VOCAB, EMB], F32R, kind="ExternalInput")
    Wi = nc.dram_tensor("Wi", [EMB, G3], F32R, kind="ExternalInput")
    Wf = nc.dram_tensor("Wf", [EMB, G3], F32R, kind="ExternalInput")
    Wr = nc.dram_tensor("Wr", [HID, G3], BF16, kind="ExternalInput")
    bias_b = nc.dram_tensor("bias_b", [128, G3], F32, kind="ExternalInput")
    bias_f = nc.dram_tensor("bias_f", [128, G3], F32, kind="ExternalInput")
    identf = nc.dram_tensor("identf", [128, 128], F32R, kind="ExternalInput")
    out = nc.dram_tensor("out", [BLOC, 2 * HID], F32, kind="ExternalOutput")

    XTd = nc.dram_tensor("XTd", [MT, 128, EMB], F32R)      # internal
    xgd = nc.dram_tensor("xgd", [NTOK, G3], BF16)         # internal

    with tile.TileContext(nc) as tc:
        # ---------------- pass A: gather + transpose ----------------
        with tc.tile_pool(name="pa", bufs=2) as pa, \
             tc.tile_pool(name="pa1", bufs=1) as pa1, \
             tc.tile_pool(name="pa_ps", bufs=4, space="PSUM") as pa_ps:
            ident = pa1.tile([128, 128], F32R)
            nc.sync.dma_start(out=ident[:], in_=identf[:, :])
            for m in range(MT):
                idx_sb = pa.tile([128, 1], I32, tag="idx")
                nc.sync.dma_start(out=idx_sb[:], in_=tok[m * 128:(m + 1) * 128, :])
                x_sb = pa.tile([128, EMB], F32R, tag="x")
                nc.gpsimd.indirect_dma_start(
                    out=x_sb[:], out_offset=None, in_=table[:, :],
                    in_offset=bass.IndirectOffsetOnAxis(ap=idx_sb[:, :1], axis=0))
                xt_sb = pa.tile([128, EMB], F32R, tag="xt")
                for q in range(KT_E):
                    t_ps = pa_ps.tile([128, 128], F32R, space="PSUM", tag="tps")
                    nc.tensor.transpose(out=t_ps[:], in_=x_sb[:, 128 * q:128 * (q + 1)],
                                        identity=ident[:])
                    nc.vector.tensor_copy(xt_sb[:, 128 * q:128 * (q + 1)], t_ps[:])
                nc.sync.dma_start(out=XTd[m, :, :], in_=xt_sb[:])

        tc.strict_bb_all_engine_barrier()
        # ---------------- phase 1: input projection ----------------
        with tc.tile_pool(name="p1w", bufs=2) as p1w, \
             tc.tile_pool(name="p1wf", bufs=1) as p1wf, \
             tc.tile_pool(name="p1", bufs=2) as p1, \
             tc.tile_pool(name="p1s", bufs=1) as p1s, \
             tc.tile_pool(name="p1_ps", bufs=2, space="PSUM") as p1_ps:
            xt0_sb = p1s.tile([128, EMB], F32R)
            nc.sync.dma_start(out=xt0_sb[:], in_=XTd[0, :, :])
            for blk in range(NG):
                cs = slice(GC * blk, GC * (blk + 1))
                wi_sb = p1w.tile([128, KT_E, GC], F32R, tag="wi")
                nc.sync.dma_start(
                    out=wi_sb[:],
                    in_=Wi[:, cs].rearrange("(k p) c -> p k c", p=128))
                wf_sb = p1wf.tile([128, KT_E, GC], F32R, tag="wf")
                nc.sync.dma_start(
                    out=wf_sb[:],
                    in_=Wf[:, cs].rearrange("(k p) c -> p k c", p=128))
                bia_sb = p1.tile([128, GC], F32, tag="bia")
                nc.sync.dma_start(out=bia_sb[:], in_=bias_b[:, cs])
                for m in range(MT):
                    xt_sb = p1.tile([128, EMB], F32R, tag="xtl")
                    nc.sync.dma_start(out=xt_sb[:], in_=XTd[m, :, :])
                    ps = p1_ps.tile([128, GC], F32, space="PSUM", tag="ps")
                    for c in range(3):
                        for k in range(KT_E):
                            nc.tensor.matmul(
                                ps[:, 512 * c:512 * (c + 1)],
                                lhsT=xt_sb[:, 128 * k:128 * (k + 1)],
                                rhs=wi_sb[:, k, 512 * c:512 * (c + 1)],
                                start=(k == 0), stop=(k == KT_E - 1))
                    xg_sb = p1.tile([128, GC], BF16, tag="xg")
                    nc.vector.tensor_add(xg_sb[:], ps[:], bia_sb[:])
                    nc.sync.dma_start(out=xgd[m * 128:(m + 1) * 128, cs], in_=xg_sb[:])
                # forward cell for this block (tokens 0..32 = original last step)
                psf = p1_ps.tile([128, GC], F32, space="PSUM", tag="ps")
                for c in range(3):
                    for k in range(KT_E):
                        nc.tensor.matmul(
                            psf[0:BLOC, 512 * c:512 * (c + 1)],
                            lhsT=xt0_sb[:, 128 * k:128 * k + BLOC],
                            rhs=wf_sb[:, k, 512 * c:512 * (c + 1)],
                            start=(k == 0), stop=(k == KT_E - 1))
                bif_sb = p1s.tile([BLOC, GC], F32, tag="bif")
                nc.sync.dma_start(out=bif_sb[:], in_=bias_f[0:BLOC, cs])
                gf = p1s.tile([BLOC, GC], F32, tag="gf")
                nc.vector.tensor_add(gf[:], psf[0:BLOC, :], bif_sb[:])
                af = p1s.tile([BLOC, HG], F32, tag="af")
                bf = p1s.tile([BLOC, HG], F32, tag="bff")
                cf = p1s.tile([BLOC, HG], F32, tag="cf")
                nc.scalar.activation(af[:], gf[:, 0:HG],
                                     mybir.ActivationFunctionType.Sigmoid)
                nc.scalar.activation(bf[:], gf[:, HG:2 * HG],
                                     mybir.ActivationFunctionType.Tanh)
                nc.scalar.activation(cf[:], gf[:, 2 * HG:3 * HG],
                                     mybir.ActivationFunctionType.Sigmoid)
                nc.vector.tensor_mul(af[:], af[:], bf[:])
                nc.scalar.activation(af[:], af[:],
                                     mybir.ActivationFunctionType.Tanh)
                nc.vector.tensor_mul(af[:], cf[:], af[:])
                nc.sync.dma_start(out=out[:, HG * blk:HG * (blk + 1)], in_=af[:])

        tc.strict_bb_all_engine_barrier()
        # ---------------- phase R: recurrence ----------------
        with tc.tile_pool(name="prw", bufs=1) as prw, \
             tc.tile_pool(name="pr", bufs=2) as pr, \
             tc.tile_pool(name="pr1", bufs=1) as pr1, \
             tc.tile_pool(name="prh", bufs=8) as prh, \
             tc.tile_pool(name="pr_ps", bufs=2, space="PSUM") as pr_ps, \
             tc.tile_pool(name="prt_ps", bufs=2, space="PSUM") as prt_ps:
            wr_sb = prw.tile([128, KT_H, G3], BF16)
            nc.sync.dma_start(
                out=wr_sb[:], in_=Wr[:, :].rearrange("(k p) c -> p k c", p=128))
            identb = pr1.tile([128, 128], BF16)
            make_identity(nc, identb[:])

            a_t = pr1.tile([128, HG], F32)
            b_t = pr1.tile([128, HG], F32)

            def load_xg(s):
                xg_sb = pr.tile([128, GC], BF16, tag="xgs")
                for j in range(NG):
                    nc.sync.dma_start(
                        out=xg_sb[BLOC * j:BLOC * (j + 1), :],
                        in_=xgd[BLOC * s:BLOC * (s + 1), GC * j:GC * (j + 1)])
                return xg_sb

            def act_and_transpose(gi_ap, gg_ap, go_ap, store_out=False):
                """gi/gg/go: [128, HG] gate APs; returns list of 4 hT chunk
                tiles [128,128] (hT[c][:, 32j:32j+32] = k-tile 4j+c)."""
                nc.scalar.activation(a_t[:], gi_ap,
                                     mybir.ActivationFunctionType.Sigmoid)
                nc.scalar.activation(b_t[:], gg_ap,
                                     mybir.ActivationFunctionType.Tanh)
                nc.vector.tensor_mul(a_t[:], a_t[:], b_t[:])      # u = sig(i)*tanh(g)
                nc.scalar.activation(a_t[:], a_t[:],
                                     mybir.ActivationFunctionType.Tanh)  # v
                nc.scalar.activation(b_t[:], go_ap,
                                     mybir.ActivationFunctionType.Sigmoid)  # c
                if store_out:
                    h_t = pr.tile([128, HG], F32, tag="hfin", bufs=1)
                    nc.vector.tensor_mul(h_t[:], b_t[:], a_t[:])
                    for j in range(NG):
                        nc.sync.dma_start(
                            out=out[:, HID + HG * j:HID + HG * (j + 1)],
                            in_=h_t[BLOC * j:BLOC * (j + 1), :])
                    return None
                # chunk-pipelined: mul -> PE transpose -> copy per 128-col chunk
                hTs = []
                for q in range(NG):
                    h_q = pr.tile([128, 128], BF16, tag="h", bufs=3)
                    nc.vector.tensor_mul(h_q[:], b_t[:, 128 * q:128 * (q + 1)],
                                         a_t[:, 128 * q:128 * (q + 1)])
                    t_ps = prt_ps.tile([128, 128], BF16, space="PSUM", tag="tps")
                    nc.tensor.transpose(out=t_ps[:], in_=h_q[:],
                                        identity=identb[:])
                    hT_q = prh.tile([128, 128], BF16, tag="hT")
                    nc.vector.tensor_copy(hT_q[:], t_ps[:])
                    hTs.append(hT_q)
                return hTs

            # step 0: h=0 -> gates are just xg
            xg0 = load_xg(0)
            hT = act_and_transpose(xg0[:, 0:HG], xg0[:, HG:2 * HG],
                                   xg0[:, 2 * HG:3 * HG])

            for s in range(1, n_steps):
                xg_sb = load_xg(s)
                # one PSUM tile per gate bank so banks don't serialize on the
                # DVE adds (Tile psum deps are tile-granular)
                ps_b = []
                for c in range(3):
                    ps_c = pr_ps.tile([128, 512], F32, space="PSUM",
                                      tag=f"gps{c}")
                    for k in range(KT_H):
                        lhs = hT[k % NG][:, BLOC * (k // NG):BLOC * (k // NG) + BLOC]
                        for j in range(NG):
                            nc.tensor.matmul(
                                ps_c[BLOC * j:BLOC * (j + 1), :],
                                lhsT=lhs,
                                rhs=wr_sb[:, k, GC * j + 512 * c:GC * j + 512 * (c + 1)],
                                start=(k == 0), stop=(k == KT_H - 1),
                                tile_position=(0, BLOC * j),
                                skip_group_check=True)
                    # fold xg into this bank as soon as its accumulation is done
                    nc.vector.tensor_add(
                        ps_c[:], ps_c[:], xg_sb[:, 512 * c:512 * (c + 1)])
                    ps_b.append(ps_c)
                hT = act_and_transpose(ps_b[0][:], ps_b[1][:], ps_b[2][:],
                                       store_out=(s == n_steps - 1))
    nc.compile()
    return nc


_BUILT = {}


def _get_built(n_steps=None):
    key = n_steps or N_STEPS
    if key not in _BUILT:
        _BUILT[key] = build(key)
    return _BUILT[key]


def _perm():
    """Row permutation taking PyTorch (i|f|g|o)*2048 rows to 4 groups of
    (i|g|o)*512."""
    p = []
    for j in range(NG):
        for base in (0, 2 * HID, 3 * HID):  # i, g, o blocks
            p.extend(range(base + HG * j, base + HG * (j + 1)))
    return np.array(p)


def prep_inputs(inputs, embed_table, W_ih_f, W_hh_f, b_ih_f, b_hh_f,
                W_ih_b, W_hh_b, b_ih_b, b_hh_b):
    perm = _perm()
    idx = np.asarray(inputs)
    idx = np.where(idx > VOCAB, 0, idx).astype(np.int64)
    idx = np.clip(idx, 0, VOCAB - 1).astype(np.int32)

    Wi_p = np.ascontiguousarray(np.asarray(W_ih_b)[perm].T.astype(np.float32))
    Wf_p = np.ascontiguousarray(np.asarray(W_ih_f)[perm].T.astype(np.float32))
    Wr_p = np.ascontiguousarray(
        np.asarray(W_hh_b)[perm].T.astype(ml_dtypes.bfloat16))
    bb = (np.asarray(b_ih_b) + np.asarray(b_hh_b))[perm].astype(np.float32)
    bf = (np.asarray(b_ih_f) + np.asarray(b_hh_f))[perm].astype(np.float32)
    bias_b_t = np.ascontiguousarray(np.broadcast_to(bb, (128, G3)))
    bias_f_t = np.ascontiguousarray(np.broadcast_to(bf, (128, G3)))
    table = np.ascontiguousarray(np.asarray(embed_table, dtype=np.float32))
    identf = np.eye(128, dtype=np.float32)

    in_maps = []
    for c in range(NCORES):
        sl = idx[BLOC * c:BLOC * (c + 1)]          # [32, 128]
        tok = np.ascontiguousarray(sl[:, ::-1].T.reshape(NTOK, 1))  # t-major rev
        in_maps.append({
            "tok": tok, "table": table, "Wi": Wi_p, "Wf": Wf_p, "Wr": Wr_p,
            "bias_b": bias_b_t, "bias_f": bias_f_t, "identf": identf,
        })
    return in_maps


def kernel(**inputs) -> np.ndarray:
    from concourse.bass_utils import run_bass_kernel_spmd
    nc = _get_built()
    in_maps = prep_inputs(**inputs)
    res = run_bass_kernel_spmd(nc, in_maps, core_ids=list(range(NCORES)))
    return np.concatenate([res.results[c]["out"] for c in range(NCORES)], axis=0)
